# revision 2
# baseline (speedup 1.0000x reference)
"""Trainium2 Bass kernel for nn_CrossScalePeriodicFeatureAggregator.

Reference computation (per expert e with patch size p_e, L_e = 336 / p_e):
    h = einsum('nld,pd->nlp', xs_e, W_e) + b_e      # [128, L_e, p_e*512]
    h -> reshape [128, 336, 512]                     # seq-stitch
    proj = h @ Wp.T + bp                             # shared projection
    out[batch_index] += gate * proj                  # gated scatter-combine

Kernel strategy (8 cores, SPMD, one uniform program):
  * Algebraic fusion: the chained matmuls collapse into one. For output
    position s = l*p_e + q:  out[n, s, :] = x[n, l, :] @ WF_e[q]  where
    WF_e[q] = W_e[q*512:(q+1)*512, :].T @ Wp.T   (precomputed on host).
    Halves device FLOPs (90 GF instead of 180 GF).
  * Gates folded into x rows on host (mathematically identical).
  * 2D sharding: core c = (rs, js) with rs = c//4 row-shard, js = c%4
    job-shard. Every core takes rows [64*rs, 64*rs+64) of every expert and
    a quarter of each expert's q-jobs ({1,2,3,6} jobs of experts {0,1,2,3}).
    All 8 cores see an IDENTICAL program shape (12 weight slots with tile
    counts [42,21,21,14,14,14,7,7,7,7,7,7]); only the bound data differs.
  * Fused weights are SBUF-RESIDENT: 12 slots x [128, 2048] bf16 = 48 KiB
    per partition, loaded once before the compute loop. Zero weight DMA in
    steady state (vs 48 MiB/core/iter in the streamed version).
  * Everything bf16 (x, WF, outputs): measured max-rel 2.4e-3 vs the fp32
    reference (tolerance 2e-2); bf16 matmul runs 1 cycle/row like fp32r,
    but halves SBUF footprint and store traffic.
  * Device writes per-unit projections out[:, u*512:(u+1)*512] (token tile
    u, partition = token%128); host de-interleaves and does the gated
    scatter-combine.

Per-core steady-state: 672 matmuls (213 ns each => ~143 us PE-bound),
22 MB bf16 stores (~61 us DMA, overlapped), 168 PSUM evictions split
across DVE and Act (~52 us each, overlapped).
"""
import numpy as np

PATCH = [4, 8, 12, 24]
SEQ = 336
D = 512
NE = 4
BATCH = 256
ROWS_PER_EXPERT = 128
N_CORES = 8
RS = 2                                   # row shards
JS = 4                                   # job shards
ROWS = ROWS_PER_EXPERT // RS             # 64 rows per core per expert
L = [SEQ // p for p in PATCH]            # [84, 42, 28, 14]
T = [ROWS * l for l in L]                # tokens/core/expert: [5376, 2688, 1792, 896]
NTILE = [t // 128 for t in T]            # token tiles: [42, 21, 14, 7] (all exact)
JOBS = [p // JS for p in PATCH]          # q-jobs per core per expert: [1, 2, 3, 6]
NSLOT = sum(JOBS)                        # 12 weight slots per core
KC = 4                                   # contraction chunks of 128
SLOT_E = [e for e in range(NE) for _ in range(JOBS[e])]   # slot -> expert
NUNIT = sum(NTILE[e] for e in SLOT_E)    # 168 output tiles per core

_CACHED = {}


def _slot_q(js, s):
    """q index handled by slot s on job-shard js (host-side data mapping)."""
    e = SLOT_E[s]
    base = sum(JOBS[:e])
    return js * JOBS[e] + (s - base)


def _build_nc(loop_n=0, internal_x=False, internal_wf=False, internal_out=False,
              spool_bufs=3, chunk=7, evict="alt"):
    """loop_n>0 wraps the compute body in a hardware For_i loop (differential
    HW timing); internal_* source/sink tensors in internal DRAM so host
    transfer stays tiny (timing builds only)."""
    import concourse.mybir as mybir
    from concourse import bacc
    from concourse.tile import TileContext

    bf16 = mybir.dt.bfloat16
    f32 = mybir.dt.float32

    nc = bacc.Bacc("TRN2", target_bir_lowering=False, debug=False,
                   num_devices=N_CORES)
    xkind = {} if internal_x else {"kind": "ExternalInput"}
    wkind = {} if internal_wf else {"kind": "ExternalInput"}
    xt = [nc.dram_tensor(f"xt{e}", [128, KC * T[e]], bf16, **xkind)
          for e in range(NE)]
    wf = nc.dram_tensor("wf", [NSLOT, 128, KC * D], bf16, **wkind)
    if internal_out:
        out = nc.dram_tensor("out", [128, NUNIT * D], bf16)
        tiny = nc.dram_tensor("tiny", [128, D], bf16, kind="ExternalOutput")
    else:
        out = nc.dram_tensor("out", [128, NUNIT * D], bf16,
                             kind="ExternalOutput")

    with TileContext(nc) as tc:
        with (
            tc.tile_pool(name="xpool", bufs=1) as xpool,
            tc.tile_pool(name="wpool", bufs=1) as wpool,
            tc.tile_pool(name="spool", bufs=spool_bufs) as spool,
            tc.tile_pool(name="ppool", bufs=8, space="PSUM") as ppool,
        ):
            # resident inputs: x (84 KiB/partition) + weights (48 KiB/part)
            xtiles = []
            for e in range(NE):
                t = xpool.tile([128, KC * T[e]], bf16, tag=f"xt{e}")
                nc.sync.dma_start(t[:], xt[e].ap())
                xtiles.append(t)
            wtiles = []
            for s in range(NSLOT):
                t = wpool.tile([128, KC * D], bf16, tag=f"wt{s}")
                nc.sync.dma_start(t[:], wf.ap()[s])
                wtiles.append(t)

            state = {"flip": 0}

            def body():
                u = 0
                for s in range(NSLOT):
                    e = SLOT_E[s]
                    for mt in range(NTILE[e]):
                        if u % chunk == 0:
                            st = spool.tile([128, chunk * D], bf16, tag="st")
                        ps = ppool.tile([128, D], f32)
                        for k in range(KC):
                            nc.tensor.matmul(
                                ps[:],
                                xtiles[e][:, k * T[e] + 128 * mt:
                                          k * T[e] + 128 * (mt + 1)],
                                wtiles[s][:, k * D:(k + 1) * D],
                                start=(k == 0), stop=(k == KC - 1),
                            )
                        dst = st[:, (u % chunk) * D:(u % chunk + 1) * D]
                        if evict == "alt":
                            eng = nc.vector if state["flip"] % 2 else nc.scalar
                        elif evict == "dve":
                            eng = nc.vector
                        else:
                            eng = nc.scalar
                        eng.tensor_copy(dst, ps[:]) if eng is nc.vector \
                            else eng.copy(dst, ps[:])
                        state["flip"] += 1
                        u += 1
                        if u % chunk == 0:
                            nc.gpsimd.dma_start(
                                out.ap()[:, (u - chunk) * D:u * D], st[:])

            if loop_n > 0:
                with tc.For_i(0, loop_n, 1):
                    body()
            else:
                body()
            if internal_out:
                nc.sync.dma_start(tiny.ap(), xtiles[0][:, :D])
    nc.compile()
    return nc


def _get_nc():
    if "nc" not in _CACHED:
        _CACHED["nc"] = _build_nc()
    return _CACHED["nc"]


def _bf16():
    import concourse.mybir as mybir
    return mybir.dt.np(mybir.dt.bfloat16)


def _prep(xs, Ws, gates, Wp, batch_index, expert_index):
    """Host-side shard prep. Returns (in_maps, row_of_expert, g_row)."""
    bf = _bf16()
    row_of_expert = [np.nonzero(expert_index == e)[0] for e in range(NE)]
    g_row = gates[batch_index, expert_index].astype(np.float32)   # [NNZ]

    # Fused weights WF_e[q] = W_e[q*512:(q+1)*512, :].T @ Wp.T  -> [d_in, o];
    # device layout wf[slot, p, k*512+o] with d_in = 128k + p.
    wf_all = []
    for e in range(NE):
        p = PATCH[e]
        w = Ws[e].reshape(p, D, D)                     # [q, m, d_in]
        WF = np.einsum("qmd,om->qdo", w, Wp, optimize=True)   # [q, d_in, o]
        wf_all.append(np.ascontiguousarray(
            WF.reshape(p, KC, 128, D).transpose(0, 2, 1, 3)   # [q, p128, k, o]
              .reshape(p, 128, KC * D)).astype(bf))

    in_maps = []
    for c in range(N_CORES):
        rs, js = c // JS, c % JS
        m = {}
        for e in range(NE):
            rows = slice(rs * ROWS, (rs + 1) * ROWS)
            gr = g_row[row_of_expert[e][rows]]
            x = xs[e][rows] * gr[:, None, None]        # [64, L, 512]
            x = x.reshape(T[e], D)                     # tokens
            # xt[p, k*T + t] = x[t, 128k + p]
            m[f"xt{e}"] = np.ascontiguousarray(
                x.reshape(T[e], KC, 128).transpose(2, 1, 0)
                 .reshape(128, KC * T[e])).astype(bf)
        m["wf"] = np.stack([wf_all[SLOT_E[s]][_slot_q(js, s)]
                            for s in range(NSLOT)])
        in_maps.append(m)
    return in_maps, row_of_expert, g_row


def _combine(results, row_of_expert, batch_index):
    """De-interleave per-unit device outputs and gated-combine per batch."""
    combined = np.zeros((BATCH, SEQ, D), np.float32)
    for e in range(NE):
        p, l = PATCH[e], L[e]
        # full_e[row, l, q, o] accumulated from all (core, slot) pairs
        full = np.empty((ROWS_PER_EXPERT, l, p, D), np.float32)
        for c in range(N_CORES):
            rs, js = c // JS, c % JS
            res = results[c]["out"]                    # [128, NUNIT*D] bf16
            u0 = 0
            for s in range(NSLOT):
                nt = NTILE[SLOT_E[s]]
                if SLOT_E[s] == e:
                    q = _slot_q(js, s)
                    blk = np.asarray(
                        res[:, u0 * D:(u0 + nt) * D], np.float32)
                    # [j, mt, o] -> token t = mt*128 + j
                    tok = blk.reshape(128, nt, D).transpose(1, 0, 2) \
                             .reshape(nt * 128, D)     # [T_e, o]
                    full[rs * ROWS:(rs + 1) * ROWS, :, q, :] = \
                        tok.reshape(ROWS, l, D)
                u0 += nt
        full = full.reshape(ROWS_PER_EXPERT, SEQ, D)
        bids = batch_index[row_of_expert[e]]
        if len(np.unique(bids)) == len(bids):
            combined[bids] += full
        else:
            np.add.at(combined, bids, full)
    return combined


def kernel(xs0, xs1, xs2, xs3, gates, W0, b0, W1, b1, W2, b2, W3, b3, Wp, bp,
           batch_index, expert_index, _collect_results=None):
    from concourse.bass_utils import run_bass_kernel_spmd

    xs = [np.asarray(x, np.float32) for x in (xs0, xs1, xs2, xs3)]
    Ws = [np.asarray(w, np.float32) for w in (W0, W1, W2, W3)]
    bs = [np.asarray(b, np.float32) for b in (b0, b1, b2, b3)]
    gates = np.asarray(gates, np.float32)
    Wp = np.asarray(Wp, np.float32)
    bp = np.asarray(bp, np.float32)
    batch_index = np.asarray(batch_index)
    expert_index = np.asarray(expert_index)

    in_maps, row_of_expert, g_row = _prep(xs, Ws, gates, Wp,
                                          batch_index, expert_index)
    nc = _get_nc()
    res = run_bass_kernel_spmd(nc, in_maps, list(range(N_CORES)))
    if _collect_results is not None:
        _collect_results.append(res)

    combined = _combine(res.results, row_of_expert, batch_index)

    # Bias terms (zero in this problem's inputs; handled for correctness).
    if any(np.any(b) for b in bs) or np.any(bp):
        for e in range(NE):
            p = PATCH[e]
            bF = bs[e].reshape(p, D) @ Wp.T + bp       # [q, o]
            bias_seq = np.tile(bF, (L[e], 1)).reshape(SEQ, D)
            bids = batch_index[row_of_expert[e]]
            gr = g_row[row_of_expert[e]]
            contrib = gr[:, None, None] * bias_seq[None]
            if len(np.unique(bids)) == len(bids):
                combined[bids] += contrib
            else:
                np.add.at(combined, bids, contrib)

    return combined


# revision 5
# speedup vs baseline: 1.1520x; 1.1520x over previous
"""Trainium2 Bass kernel for nn_CrossScalePeriodicFeatureAggregator.

Reference computation (per expert e with patch size p_e, L_e = 336 / p_e):
    h = einsum('nld,pd->nlp', xs_e, W_e) + b_e      # [128, L_e, p_e*512]
    h -> reshape [128, 336, 512]                     # seq-stitch
    proj = h @ Wp.T + bp                             # shared projection
    out[batch_index] += gate * proj                  # gated scatter-combine

Kernel strategy (8 cores, SPMD, one uniform program):
  * Algebraic fusion: the chained matmuls collapse into one. For output
    position s = l*p_e + q:  out[n, s, :] = x[n, l, :] @ WF_e[q]  where
    WF_e[q] = W_e[q*512:(q+1)*512, :].T @ Wp.T   (precomputed on host).
    Halves device FLOPs (90 GF instead of 180 GF).
  * Gates folded into x rows on host (mathematically identical).
  * 2D sharding: core c = (rs, js) with rs = c//4 row-shard, js = c%4
    job-shard. Every core takes rows [64*rs, 64*rs+64) of every expert and
    a quarter of each expert's q-jobs ({1,2,3,6} jobs of experts {0,1,2,3}).
    All 8 cores see an IDENTICAL program shape (12 weight slots with tile
    counts [42,21,21,14,14,14,7,7,7,7,7,7]); only the bound data differs.
  * Fused weights are SBUF-RESIDENT: 12 slots x [128, 2048] bf16 = 48 KiB
    per partition, loaded once before the compute loop. Zero weight DMA in
    steady state (vs 48 MiB/core/iter in the streamed version).
  * Everything bf16 (x, WF, outputs): measured max-rel 2.4e-3 vs the fp32
    reference (tolerance 2e-2); bf16 matmul runs 1 cycle/row like fp32r,
    but halves SBUF footprint and store traffic.
  * Device writes per-unit projections out[:, u*512:(u+1)*512] (token tile
    u, partition = token%128); host de-interleaves and does the gated
    scatter-combine.

Per-core steady-state: 672 matmuls (213 ns each => ~143 us PE-bound),
22 MB bf16 stores (~61 us DMA, overlapped), 168 PSUM evictions split
across DVE and Act (~52 us each, overlapped).
"""
import numpy as np

PATCH = [4, 8, 12, 24]
SEQ = 336
D = 512
NE = 4
BATCH = 256
ROWS_PER_EXPERT = 128
N_CORES = 8
RS = 2                                   # row shards
JS = 4                                   # job shards
ROWS = ROWS_PER_EXPERT // RS             # 64 rows per core per expert
L = [SEQ // p for p in PATCH]            # [84, 42, 28, 14]
T = [ROWS * l for l in L]                # tokens/core/expert: [5376, 2688, 1792, 896]
NTILE = [t // 128 for t in T]            # token tiles: [42, 21, 14, 7] (all exact)
JOBS = [p // JS for p in PATCH]          # q-jobs per core per expert: [1, 2, 3, 6]
NSLOT = sum(JOBS)                        # 12 weight slots per core
KC = 4                                   # contraction chunks of 128
SLOT_E = [e for e in range(NE) for _ in range(JOBS[e])]   # slot -> expert
NUNIT = sum(NTILE[e] for e in SLOT_E)    # 168 output tiles per core

_CACHED = {}


def _slot_q(js, s):
    """q index handled by slot s on job-shard js (host-side data mapping)."""
    e = SLOT_E[s]
    base = sum(JOBS[:e])
    return js * JOBS[e] + (s - base)


def _build_nc(loop_n=0, internal_x=False, internal_wf=False, internal_out=False,
              spool_bufs=3, chunk=7, evict="alt",
              store_engs=("gpsimd", "sync", "scalar")):
    """loop_n>0 wraps the compute body in a hardware For_i loop (differential
    HW timing); internal_* source/sink tensors in internal DRAM so host
    transfer stays tiny (timing builds only)."""
    import concourse.mybir as mybir
    from concourse import bacc
    from concourse.tile import TileContext

    bf16 = mybir.dt.bfloat16
    f32 = mybir.dt.float32

    nc = bacc.Bacc("TRN2", target_bir_lowering=False, debug=False,
                   num_devices=N_CORES)
    xkind = {} if internal_x else {"kind": "ExternalInput"}
    wkind = {} if internal_wf else {"kind": "ExternalInput"}
    xt = [nc.dram_tensor(f"xt{e}", [128, KC * T[e]], bf16, **xkind)
          for e in range(NE)]
    wf = nc.dram_tensor("wf", [NSLOT, 128, KC * D], bf16, **wkind)
    if internal_out:
        out = nc.dram_tensor("out", [128, NUNIT * D], bf16)
        tiny = nc.dram_tensor("tiny", [128, D], bf16, kind="ExternalOutput")
    else:
        out = nc.dram_tensor("out", [128, NUNIT * D], bf16,
                             kind="ExternalOutput")

    with TileContext(nc) as tc:
        with (
            tc.tile_pool(name="xpool", bufs=1) as xpool,
            tc.tile_pool(name="wpool", bufs=1) as wpool,
            tc.tile_pool(name="spool", bufs=spool_bufs) as spool,
            tc.tile_pool(name="ppool", bufs=8, space="PSUM") as ppool,
        ):
            # resident inputs: x (84 KiB/partition) + weights (48 KiB/part)
            xtiles = []
            for e in range(NE):
                t = xpool.tile([128, KC * T[e]], bf16, tag=f"xt{e}")
                nc.sync.dma_start(t[:], xt[e].ap())
                xtiles.append(t)
            wtiles = []
            for s in range(NSLOT):
                t = wpool.tile([128, KC * D], bf16, tag=f"wt{s}")
                nc.sync.dma_start(t[:], wf.ap()[s])
                wtiles.append(t)

            # stores round-robin across 3 DMA rings: a single ring caps at
            # ~73 GB/s and serializes behind the 22 MB/iter store stream
            # (measured: 1 ring 309 us, >=2 rings 233 us)
            sengs = [getattr(nc, n) for n in store_engs]
            state = {"flip": 0, "dma": 0}

            def body():
                u = 0
                for s in range(NSLOT):
                    e = SLOT_E[s]
                    for mt in range(NTILE[e]):
                        if u % chunk == 0:
                            st = spool.tile([128, chunk * D], bf16, tag="st")
                        ps = ppool.tile([128, D], f32)
                        for k in range(KC):
                            nc.tensor.matmul(
                                ps[:],
                                xtiles[e][:, k * T[e] + 128 * mt:
                                          k * T[e] + 128 * (mt + 1)],
                                wtiles[s][:, k * D:(k + 1) * D],
                                start=(k == 0), stop=(k == KC - 1),
                            )
                        dst = st[:, (u % chunk) * D:(u % chunk + 1) * D]
                        if evict == "alt":
                            eng = nc.vector if state["flip"] % 2 else nc.scalar
                        elif evict == "dve":
                            eng = nc.vector
                        else:
                            eng = nc.scalar
                        eng.tensor_copy(dst, ps[:]) if eng is nc.vector \
                            else eng.copy(dst, ps[:])
                        state["flip"] += 1
                        u += 1
                        if u % chunk == 0:
                            eng = sengs[state["dma"] % len(sengs)]
                            eng.dma_start(
                                out.ap()[:, (u - chunk) * D:u * D], st[:])
                            state["dma"] += 1

            if loop_n > 0:
                with tc.For_i(0, loop_n, 1):
                    body()
            else:
                body()
            if internal_out:
                nc.sync.dma_start(tiny.ap(), xtiles[0][:, :D])
    nc.compile()
    return nc


def _get_nc():
    if "nc" not in _CACHED:
        _CACHED["nc"] = _build_nc()
    return _CACHED["nc"]


def _bf16():
    import concourse.mybir as mybir
    return mybir.dt.np(mybir.dt.bfloat16)


def _prep(xs, Ws, gates, Wp, batch_index, expert_index):
    """Host-side shard prep. Returns (in_maps, row_of_expert, g_row)."""
    bf = _bf16()
    row_of_expert = [np.nonzero(expert_index == e)[0] for e in range(NE)]
    g_row = gates[batch_index, expert_index].astype(np.float32)   # [NNZ]

    # Fused weights WF_e[q] = W_e[q*512:(q+1)*512, :].T @ Wp.T  -> [d_in, o];
    # device layout wf[slot, p, k*512+o] with d_in = 128k + p.
    wf_all = []
    for e in range(NE):
        p = PATCH[e]
        w = Ws[e].reshape(p, D, D)                     # [q, m, d_in]
        WF = np.einsum("qmd,om->qdo", w, Wp, optimize=True)   # [q, d_in, o]
        wf_all.append(np.ascontiguousarray(
            WF.reshape(p, KC, 128, D).transpose(0, 2, 1, 3)   # [q, p128, k, o]
              .reshape(p, 128, KC * D)).astype(bf))

    in_maps = []
    for c in range(N_CORES):
        rs, js = c // JS, c % JS
        m = {}
        for e in range(NE):
            rows = slice(rs * ROWS, (rs + 1) * ROWS)
            gr = g_row[row_of_expert[e][rows]]
            x = xs[e][rows] * gr[:, None, None]        # [64, L, 512]
            x = x.reshape(T[e], D)                     # tokens
            # xt[p, k*T + t] = x[t, 128k + p]
            m[f"xt{e}"] = np.ascontiguousarray(
                x.reshape(T[e], KC, 128).transpose(2, 1, 0)
                 .reshape(128, KC * T[e])).astype(bf)
        m["wf"] = np.stack([wf_all[SLOT_E[s]][_slot_q(js, s)]
                            for s in range(NSLOT)])
        in_maps.append(m)
    return in_maps, row_of_expert, g_row


def _combine(results, row_of_expert, batch_index):
    """De-interleave per-unit device outputs and gated-combine per batch."""
    combined = np.zeros((BATCH, SEQ, D), np.float32)
    for e in range(NE):
        p, l = PATCH[e], L[e]
        # full_e[row, l, q, o] accumulated from all (core, slot) pairs
        full = np.empty((ROWS_PER_EXPERT, l, p, D), np.float32)
        for c in range(N_CORES):
            rs, js = c // JS, c % JS
            res = results[c]["out"]                    # [128, NUNIT*D] bf16
            u0 = 0
            for s in range(NSLOT):
                nt = NTILE[SLOT_E[s]]
                if SLOT_E[s] == e:
                    q = _slot_q(js, s)
                    blk = np.asarray(
                        res[:, u0 * D:(u0 + nt) * D], np.float32)
                    # [j, mt, o] -> token t = mt*128 + j
                    tok = blk.reshape(128, nt, D).transpose(1, 0, 2) \
                             .reshape(nt * 128, D)     # [T_e, o]
                    full[rs * ROWS:(rs + 1) * ROWS, :, q, :] = \
                        tok.reshape(ROWS, l, D)
                u0 += nt
        full = full.reshape(ROWS_PER_EXPERT, SEQ, D)
        bids = batch_index[row_of_expert[e]]
        if len(np.unique(bids)) == len(bids):
            combined[bids] += full
        else:
            np.add.at(combined, bids, full)
    return combined


def kernel(xs0, xs1, xs2, xs3, gates, W0, b0, W1, b1, W2, b2, W3, b3, Wp, bp,
           batch_index, expert_index, _collect_results=None):
    from concourse.bass_utils import run_bass_kernel_spmd

    xs = [np.asarray(x, np.float32) for x in (xs0, xs1, xs2, xs3)]
    Ws = [np.asarray(w, np.float32) for w in (W0, W1, W2, W3)]
    bs = [np.asarray(b, np.float32) for b in (b0, b1, b2, b3)]
    gates = np.asarray(gates, np.float32)
    Wp = np.asarray(Wp, np.float32)
    bp = np.asarray(bp, np.float32)
    batch_index = np.asarray(batch_index)
    expert_index = np.asarray(expert_index)

    in_maps, row_of_expert, g_row = _prep(xs, Ws, gates, Wp,
                                          batch_index, expert_index)
    nc = _get_nc()
    res = run_bass_kernel_spmd(nc, in_maps, list(range(N_CORES)))
    if _collect_results is not None:
        _collect_results.append(res)

    combined = _combine(res.results, row_of_expert, batch_index)

    # Bias terms (zero in this problem's inputs; handled for correctness).
    if any(np.any(b) for b in bs) or np.any(bp):
        for e in range(NE):
            p = PATCH[e]
            bF = bs[e].reshape(p, D) @ Wp.T + bp       # [q, o]
            bias_seq = np.tile(bF, (L[e], 1)).reshape(SEQ, D)
            bids = batch_index[row_of_expert[e]]
            gr = g_row[row_of_expert[e]]
            contrib = gr[:, None, None] * bias_seq[None]
            if len(np.unique(bids)) == len(bids):
                combined[bids] += contrib
            else:
                np.add.at(combined, bids, contrib)

    return combined
